# revision 1
# baseline (speedup 1.0000x reference)
"""Trainium2 Bass kernel for nn_CausE (embedding_lookup).

Computation (per batch element b):
    u = user_e[user[b]]; v = item_e_c[item[b]]
    s = dot(u, v)
    t = exp(s) if s <= 0 else s + 1          (== exp(min(s,0)) + max(s,0))
    x = t * pop_item[item[b]] ** 0.5
    out = log_sigmoid(x) + user_b[user[b]] + item_b[item[b]]
        = (user_b + item_b) - ln(1 + exp(-x))

Sharding / layout:
  * Batch elements are routed to 8 cores by user-index range (user // 125000),
    so each core holds only a 125K-row slice of the user table.
  * Tables are augmented host-side (pure concatenation) into 512-byte rows:
        u_tab[r] = [user_e row (64 f32), user_b, 63 pad]
        v_tab[r] = [item_e_c row (64 f32), pop_item, item_b, 62 pad]
    so ONE dma_gather descriptor per element per side fetches everything.
    (dma_gather descriptor generation on the GPSIMD Q7 cores is the
    bottleneck at ~8 ns/descriptor and is independent of row size, so the
    fat f32 rows cost nothing vs bf16 packing.)
  * dma_gather indices are int16, so within each core the elements are
    sorted by (user_bucket, item_bucket) where bucket = index >> 15; each
    of the 16 segments is gathered with bucket-base-offset table APs.
  * All gather indices live in one resident SBUF buffer (loaded once);
    results accumulate in a resident SBUF buffer, stored once at the end.
  * All transcendentals use only Exp/Ln so a single activation table serves
    the whole kernel: sqrt(p) = exp(0.5*ln(p)), softplus = ln(1+exp(.)).
"""

import os

import numpy as np

NUM_USERS = 1_000_000
NUM_ITEMS = 100_000
EMBED = 64
N_CORES = 8
UPC = NUM_USERS // N_CORES  # 125_000 user rows per core
P = 128
ROWW = 128          # f32 slots per augmented table row (512 B)
BKT = 32768         # int16 index range per bucket
T_CH = int(os.environ.get("KERNEL_TCH", "24"))   # tile columns per gather chunk
CHUNK = P * T_CH

LAST_EXEC_NS = None
LAST_RESULTS = None


def _build_program(chunks, c_all, c_out):
    """chunks: list of (n_idx, t_cols, u_base_row, v_base_row,
                        u_idx_col, v_idx_col, out_col)."""
    import concourse.bacc as bacc
    import concourse.mybir as mybir
    from concourse.tile import TileContext

    f32 = mybir.dt.float32
    i16 = mybir.dt.int16
    AF = mybir.ActivationFunctionType
    ALU = mybir.AluOpType
    X = mybir.AxisListType.X

    nc = bacc.Bacc(
        "TRN2",
        target_bir_lowering=False,
        debug=False,
        enable_asserts=False,
        num_devices=N_CORES,
    )
    u_tab = nc.dram_tensor("u_tab", [UPC, ROWW], f32, kind="ExternalInput")
    v_tab = nc.dram_tensor("v_tab", [NUM_ITEMS, ROWW], f32, kind="ExternalInput")
    idxs = nc.dram_tensor("idxs", [P, c_all], i16, kind="ExternalInput")
    outd = nc.dram_tensor("outd", [P, c_out], f32, kind="ExternalOutput")

    u_ap = u_tab.ap()
    v_ap = v_tab.ap()

    with TileContext(nc) as tc:
        with (
            tc.tile_pool(name="res", bufs=1) as rp,
            tc.tile_pool(name="gath", bufs=int(os.environ.get("KERNEL_GBUFS", "3"))) as gp,
            tc.tile_pool(name="tmp", bufs=3) as tp,
        ):
            idx_res = rp.tile([P, c_all], i16, tag="idx_res")
            nc.sync.dma_start(idx_res[:], idxs.ap())
            out_res = rp.tile([P, c_out], f32, tag="out_res")

            for (n, t, ub, vb, uc, vc, oc) in chunks:
                ug = gp.tile([P, T_CH, ROWW], f32, tag="u")
                nc.gpsimd.dma_gather(
                    out_ap=ug[:, 0:t, :],
                    in_ap=u_ap[ub : min(ub + BKT, UPC)],
                    idxs_ap=idx_res[:, uc : uc + n // 16],
                    num_idxs=n,
                    num_idxs_reg=n,
                    elem_size=ROWW,
                    elem_step=ROWW,
                    single_packet=False,
                )
                vg = gp.tile([P, T_CH, ROWW], f32, tag="v")
                nc.gpsimd.dma_gather(
                    out_ap=vg[:, 0:t, :],
                    in_ap=v_ap[vb : min(vb + BKT, NUM_ITEMS)],
                    idxs_ap=idx_res[:, vc : vc + n // 16],
                    num_idxs=n,
                    num_idxs_reg=n,
                    elem_size=ROWW,
                    elem_step=ROWW,
                    single_packet=False,
                )

                prod = tp.tile([P, T_CH, EMBED], f32, tag="prod")
                nc.vector.tensor_mul(
                    prod[:, 0:t, :], ug[:, 0:t, 0:EMBED], vg[:, 0:t, 0:EMBED]
                )
                s = tp.tile([P, T_CH], f32, tag="s")
                nc.vector.tensor_reduce(s[:, 0:t], prod[:, 0:t, :], axis=X, op=ALU.add)
                # t_ = exp(min(s,0)) + max(s,0)
                # min(s,0) = 0.5*s - |0.5*s| -- avoids TENSOR_SCALAR, which
                # measures ~8us/op on HW vs ~1us for scalar_tensor_tensor.
                a2 = tp.tile([P, T_CH], f32, tag="a2")
                nc.scalar.activation(a2[:, 0:t], s[:, 0:t], AF.Abs, scale=0.5)
                m = tp.tile([P, T_CH], f32, tag="m")
                nc.vector.scalar_tensor_tensor(
                    m[:, 0:t], s[:, 0:t], 0.5, a2[:, 0:t],
                    op0=ALU.mult, op1=ALU.subtract,
                )
                e = tp.tile([P, T_CH], f32, tag="e")
                nc.scalar.activation(e[:, 0:t], m[:, 0:t], AF.Exp)
                t_ = tp.tile([P, T_CH], f32, tag="t_")
                nc.vector.scalar_tensor_tensor(
                    t_[:, 0:t], s[:, 0:t], 0.0, e[:, 0:t], op0=ALU.max, op1=ALU.add
                )
                # w = sqrt(pop) = exp(0.5*ln(pop))
                lp = tp.tile([P, T_CH], f32, tag="lp")
                nc.scalar.activation(lp[:, 0:t], vg[:, 0:t, EMBED], AF.Ln)
                w = tp.tile([P, T_CH], f32, tag="w")
                nc.scalar.activation(w[:, 0:t], lp[:, 0:t], AF.Exp, scale=0.5)
                x = tp.tile([P, T_CH], f32, tag="x")
                nc.vector.tensor_mul(x[:, 0:t], t_[:, 0:t], w[:, 0:t])
                # softplus(-x) = ln(1 + exp(-x))
                ex = tp.tile([P, T_CH], f32, tag="ex")
                nc.scalar.activation(ex[:, 0:t], x[:, 0:t], AF.Exp, scale=-1.0)
                sp = tp.tile([P, T_CH], f32, tag="sp")
                nc.scalar.activation(sp[:, 0:t], ex[:, 0:t], AF.Ln, bias=1.0)
                # out = (user_b + item_b) - softplus(-x)
                b = tp.tile([P, T_CH], f32, tag="b")
                nc.vector.tensor_add(
                    b[:, 0:t], ug[:, 0:t, EMBED], vg[:, 0:t, EMBED + 1]
                )
                nc.vector.tensor_sub(out_res[:, oc : oc + t], b[:, 0:t], sp[:, 0:t])

            nc.sync.dma_start(outd.ap(), out_res[:])

    nc.compile()
    return nc


class _SimResults:
    def __init__(self, results):
        self.results = results
        self.exec_time_ns = None
        self.instructions_and_trace = None
        self.profile_json = None


def _run_sim(nc, in_maps):
    from concourse.bass_interp import CoreSim

    results = []
    for m in in_maps:
        sim = CoreSim(nc, require_finite=False, require_nnan=False)
        for k, v in m.items():
            sim.tensor(k)[:] = v
        sim.simulate()
        results.append({"outd": np.array(sim.tensor("outd"))})
    return _SimResults(results)


def _wrap_idx(rel16):
    """[n] int16 -> [128, n//16]: index i at (partition i%16, col i//16),
    replicated across the 8 GPSIMD core groups."""
    n = rel16.shape[0]
    w = rel16.reshape(n // 16, 16).T  # [16, n//16]
    return np.tile(w, (8, 1))


def kernel(user, item, user_e, item_e_c, user_b, item_b, pop_item, **_unused):
    global LAST_EXEC_NS, LAST_RESULTS

    user = np.asarray(user).astype(np.int64, copy=False)
    item = np.asarray(item).astype(np.int64, copy=False)
    user_e = np.ascontiguousarray(np.asarray(user_e, dtype=np.float32))
    item_e_c = np.ascontiguousarray(np.asarray(item_e_c, dtype=np.float32))
    user_b = np.asarray(user_b, dtype=np.float32).reshape(-1)
    item_b = np.asarray(item_b, dtype=np.float32).reshape(-1)
    pop_item = np.asarray(pop_item, dtype=np.float32).reshape(-1)

    batch = user.shape[0]

    # ---- route to cores by user range; sort by (ubkt, ibkt) segment --------
    core_of = (user // UPC).astype(np.int32)
    local_u = (user - core_of.astype(np.int64) * UPC).astype(np.int32)
    seg_of = ((local_u >> 15) << 2 | (item >> 15)).astype(np.int32)  # 0..15

    # per (core, seg) element lists
    order = np.lexsort((seg_of, core_of))  # grouped by core, then seg
    core_sorted = core_of[order]
    seg_sorted = seg_of[order]

    counts = np.zeros((N_CORES, 16), dtype=np.int64)
    np.add.at(counts, (core_of, seg_of), 1)

    # padded segment sizes shared by all cores (multiples of 128)
    seg_pad = ((counts.max(axis=0) + 127) // 128 * 128).astype(np.int64)

    # chunk plan (identical across cores)
    chunks_meta = []  # (seg, n, t, off_within_seg)
    seg_off = np.zeros(17, dtype=np.int64)
    for sgi in range(16):
        seg_off[sgi + 1] = seg_off[sgi] + seg_pad[sgi]
        done = 0
        while done < seg_pad[sgi]:
            n = int(min(CHUNK, seg_pad[sgi] - done))
            chunks_meta.append((sgi, n, n // P, done))
            done += n
    n_pad = int(seg_off[16])
    c_all = 2 * (n_pad // 16)
    c_out = n_pad // P

    # device chunk list with buffer offsets
    chunks = []
    ucol = 0
    vcol = n_pad // 16
    ocol = 0
    for (sgi, n, t, _off) in chunks_meta:
        ub = (sgi >> 2) * BKT
        vb = (sgi & 3) * BKT
        chunks.append((n, t, ub, vb, ucol, vcol, ocol))
        ucol += n // 16
        vcol += n // 16
        ocol += t

    # ---- augmented 512B-row tables (pure concatenation) --------------------
    v_aug = np.zeros((NUM_ITEMS, ROWW), dtype=np.float32)
    v_aug[:, :EMBED] = item_e_c
    v_aug[:, EMBED] = pop_item
    v_aug[:, EMBED + 1] = item_b

    # ---- per-core index streams -------------------------------------------
    in_maps = []
    recon = []  # per core: list of (lo, hi) element ranges in `order`
    core_starts = np.searchsorted(core_sorted, np.arange(N_CORES + 1))
    for c in range(N_CORES):
        u_aug = np.zeros((UPC, ROWW), dtype=np.float32)
        u_aug[:, :EMBED] = user_e[c * UPC : (c + 1) * UPC]
        u_aug[:, EMBED] = user_b[c * UPC : (c + 1) * UPC]

        clo, chi = core_starts[c], core_starts[c + 1]
        segs = seg_sorted[clo:chi]
        seg_bounds = np.searchsorted(segs, np.arange(17))

        u_rel = np.zeros(n_pad, dtype=np.int16)
        v_rel = np.zeros(n_pad, dtype=np.int16)
        for sgi in range(16):
            lo, hi = seg_bounds[sgi], seg_bounds[sgi + 1]
            els = order[clo + lo : clo + hi]
            base = seg_off[sgi]
            cnt = hi - lo
            u_rel[base : base + cnt] = (local_u[els] - (sgi >> 2) * BKT).astype(
                np.int16
            )
            v_rel[base : base + cnt] = (item[els] - (sgi & 3) * BKT).astype(np.int16)
            # pad slots keep 0 (valid row at the bucket base)
        idx_arr = np.empty((P, c_all), dtype=np.int16)
        idx_arr[:, : n_pad // 16] = _wrap_idx(u_rel)
        idx_arr[:, n_pad // 16 :] = _wrap_idx(v_rel)

        in_maps.append({"u_tab": u_aug, "v_tab": v_aug, "idxs": idx_arr})
        recon.append((clo, chi, seg_bounds))

    nc = _build_program(chunks, c_all, c_out)

    if os.environ.get("KERNEL_SIM", "0") == "1":
        res = _run_sim(nc, in_maps)
    else:
        from concourse.bass_utils import run_bass_kernel_spmd

        trace = os.environ.get("KERNEL_TRACE", "0") == "1"
        res = run_bass_kernel_spmd(
            nc,
            in_maps,
            core_ids=list(range(N_CORES)),
            trace=trace,
        )
    LAST_EXEC_NS = res.exec_time_ns
    LAST_RESULTS = res

    # ---- reconstruct -------------------------------------------------------
    out_full = np.empty(batch, dtype=np.float32)
    for c in range(N_CORES):
        clo, chi, seg_bounds = recon[c]
        arr = res.results[c]["outd"]  # [P, c_out]
        flat = arr.reshape(P, c_out).ravel(order="F")  # element i at i (p fastest)
        for sgi in range(16):
            lo, hi = seg_bounds[sgi], seg_bounds[sgi + 1]
            els = order[clo + lo : clo + hi]
            base = int(seg_off[sgi])
            out_full[els] = flat[base : base + (hi - lo)]
    return out_full



# revision 5
# speedup vs baseline: 3.7193x; 3.7193x over previous
"""Trainium2 Bass kernel for nn_CausE (embedding_lookup).

Computation (per batch element b):
    u = user_e[user[b]]; v = item_e_c[item[b]]
    s = dot(u, v)
    t = exp(s) if s <= 0 else s + 1          (== exp(min(s,0)) + max(s,0))
    x = t * pop_item[item[b]] ** 0.5
    out = log_sigmoid(x) + user_b[user[b]] + item_b[item[b]]
        = (user_b + item_b) - ln(1 + exp(-x))

Sharding / layout:
  * Batch elements are routed to 8 cores by user-index range (user // 125000),
    so each core holds only a 125K-row slice of the user table.
  * Tables are augmented host-side (pure concatenation) into 512-byte rows:
        u_tab[r] = [user_e row (64 f32), user_b, 63 pad]
        v_tab[r] = [item_e_c row (64 f32), pop_item, item_b, 62 pad]
    so ONE dma_gather descriptor per element per side fetches everything.
    (dma_gather descriptor generation on the GPSIMD Q7 cores is the
    bottleneck at ~8 ns/descriptor and is independent of row size, so the
    fat f32 rows cost nothing vs bf16 packing.)
  * dma_gather indices are int16, so within each core the elements are
    sorted by (user_bucket, item_bucket) where bucket = index >> 15; each
    of the 16 segments is gathered with bucket-base-offset table APs.
  * All gather indices live in one resident SBUF buffer (loaded once);
    results accumulate in a resident SBUF buffer, stored once at the end.
  * All transcendentals use only Exp/Ln so a single activation table serves
    the whole kernel: sqrt(p) = exp(0.5*ln(p)), softplus = ln(1+exp(.)).
"""

import os

import numpy as np

NUM_USERS = 1_000_000
NUM_ITEMS = 100_000
EMBED = 64
N_CORES = 8
UPC = NUM_USERS // N_CORES  # 125_000 user rows per core
P = 128
ROWW = 128          # f32 slots per augmented table row (512 B)
BKT = 32768         # int16 index range per bucket
T_CH = int(os.environ.get("KERNEL_TCH", "24"))   # tile columns per gather chunk
CHUNK = P * T_CH
N_QUEUES = int(os.environ.get("KERNEL_NQ", "4"))  # SWDGE queues (Q7 core pairs)

LAST_EXEC_NS = None
LAST_RESULTS = None


def _build_program(chunks, c_all, c_out):
    """chunks: list of (n_idx, t_cols, u_base_row, v_base_row,
                        u_idx_col, v_idx_col, out_col)."""
    import concourse.bacc as bacc
    import concourse.mybir as mybir
    from concourse.tile import TileContext

    f32 = mybir.dt.float32
    i16 = mybir.dt.int16
    AF = mybir.ActivationFunctionType
    ALU = mybir.AluOpType
    X = mybir.AxisListType.X

    nc = bacc.Bacc(
        "TRN2",
        target_bir_lowering=False,
        debug=False,
        enable_asserts=False,
        num_devices=N_CORES,
        num_swdge_queues=N_QUEUES,
    )
    u_tab = nc.dram_tensor("u_tab", [UPC, ROWW], f32, kind="ExternalInput")
    v_tab = nc.dram_tensor("v_tab", [NUM_ITEMS, ROWW], f32, kind="ExternalInput")
    idxs = nc.dram_tensor("idxs", [P, c_all], i16, kind="ExternalInput")
    outd = nc.dram_tensor("outd", [P, c_out], f32, kind="ExternalOutput")

    u_ap = u_tab.ap()
    v_ap = v_tab.ap()

    with TileContext(nc) as tc:
        with (
            tc.tile_pool(name="res", bufs=1) as rp,
            tc.tile_pool(name="gath", bufs=int(os.environ.get("KERNEL_GBUFS", "4"))) as gp,
            tc.tile_pool(name="tmp", bufs=3) as tp,
        ):
            idx_res = rp.tile([P, c_all], i16, tag="idx_res")
            nc.sync.dma_start(idx_res[:], idxs.ap())
            out_res = rp.tile([P, c_out], f32, tag="out_res")

            for ci, (n, t, ub, vb, uc, vc, oc) in enumerate(chunks):
                ug = gp.tile([P, T_CH, ROWW], f32, tag="u")
                nc.gpsimd.dma_gather(
                    out_ap=ug[:, 0:t, :],
                    in_ap=u_ap[ub : min(ub + BKT, UPC)],
                    idxs_ap=idx_res[:, uc : uc + n // 16],
                    num_idxs=n,
                    num_idxs_reg=n,
                    elem_size=ROWW,
                    elem_step=ROWW,
                    single_packet=False,
                    queue_num=(2 * ci) % N_QUEUES,
                )
                vg = gp.tile([P, T_CH, ROWW], f32, tag="v")
                nc.gpsimd.dma_gather(
                    out_ap=vg[:, 0:t, :],
                    in_ap=v_ap[vb : min(vb + BKT, NUM_ITEMS)],
                    idxs_ap=idx_res[:, vc : vc + n // 16],
                    num_idxs=n,
                    num_idxs_reg=n,
                    elem_size=ROWW,
                    elem_step=ROWW,
                    single_packet=False,
                    queue_num=(2 * ci + 1) % N_QUEUES,
                )

                prod = tp.tile([P, T_CH, EMBED], f32, tag="prod")
                nc.vector.tensor_mul(
                    prod[:, 0:t, :], ug[:, 0:t, 0:EMBED], vg[:, 0:t, 0:EMBED]
                )
                s = tp.tile([P, T_CH], f32, tag="s")
                nc.vector.tensor_reduce(s[:, 0:t], prod[:, 0:t, :], axis=X, op=ALU.add)
                # t_ = exp(min(s,0)) + max(s,0)
                # min(s,0) = 0.5*s - |0.5*s| -- avoids TENSOR_SCALAR, which
                # measures ~8us/op on HW vs ~1us for scalar_tensor_tensor.
                a2 = tp.tile([P, T_CH], f32, tag="a2")
                nc.scalar.activation(a2[:, 0:t], s[:, 0:t], AF.Abs, scale=0.5)
                m = tp.tile([P, T_CH], f32, tag="m")
                nc.vector.scalar_tensor_tensor(
                    m[:, 0:t], s[:, 0:t], 0.5, a2[:, 0:t],
                    op0=ALU.mult, op1=ALU.subtract,
                )
                e = tp.tile([P, T_CH], f32, tag="e")
                nc.scalar.activation(e[:, 0:t], m[:, 0:t], AF.Exp)
                t_ = tp.tile([P, T_CH], f32, tag="t_")
                nc.vector.scalar_tensor_tensor(
                    t_[:, 0:t], s[:, 0:t], 0.0, e[:, 0:t], op0=ALU.max, op1=ALU.add
                )
                # w = sqrt(pop) = exp(0.5*ln(pop))
                lp = tp.tile([P, T_CH], f32, tag="lp")
                nc.scalar.activation(lp[:, 0:t], vg[:, 0:t, EMBED], AF.Ln)
                w = tp.tile([P, T_CH], f32, tag="w")
                nc.scalar.activation(w[:, 0:t], lp[:, 0:t], AF.Exp, scale=0.5)
                x = tp.tile([P, T_CH], f32, tag="x")
                nc.vector.tensor_mul(x[:, 0:t], t_[:, 0:t], w[:, 0:t])
                # softplus(-x) = ln(1 + exp(-x))
                ex = tp.tile([P, T_CH], f32, tag="ex")
                nc.scalar.activation(ex[:, 0:t], x[:, 0:t], AF.Exp, scale=-1.0)
                sp = tp.tile([P, T_CH], f32, tag="sp")
                nc.scalar.activation(sp[:, 0:t], ex[:, 0:t], AF.Ln, bias=1.0)
                # out = (user_b + item_b) - softplus(-x)
                b = tp.tile([P, T_CH], f32, tag="b")
                nc.vector.tensor_add(
                    b[:, 0:t], ug[:, 0:t, EMBED], vg[:, 0:t, EMBED + 1]
                )
                nc.vector.tensor_sub(out_res[:, oc : oc + t], b[:, 0:t], sp[:, 0:t])

            nc.sync.dma_start(outd.ap(), out_res[:])

    nc.compile()
    return nc


class _SimResults:
    def __init__(self, results):
        self.results = results
        self.exec_time_ns = None
        self.instructions_and_trace = None
        self.profile_json = None


def _run_sim(nc, in_maps):
    from concourse.bass_interp import CoreSim

    results = []
    for m in in_maps:
        sim = CoreSim(nc, require_finite=False, require_nnan=False)
        for k, v in m.items():
            sim.tensor(k)[:] = v
        sim.simulate()
        results.append({"outd": np.array(sim.tensor("outd"))})
    return _SimResults(results)


def _wrap_idx(rel16):
    """[n] int16 -> [128, n//16]: index i at (partition i%16, col i//16),
    replicated across the 8 GPSIMD core groups."""
    n = rel16.shape[0]
    w = rel16.reshape(n // 16, 16).T  # [16, n//16]
    return np.tile(w, (8, 1))


def kernel(user, item, user_e, item_e_c, user_b, item_b, pop_item, **_unused):
    global LAST_EXEC_NS, LAST_RESULTS

    user = np.asarray(user).astype(np.int64, copy=False)
    item = np.asarray(item).astype(np.int64, copy=False)
    user_e = np.ascontiguousarray(np.asarray(user_e, dtype=np.float32))
    item_e_c = np.ascontiguousarray(np.asarray(item_e_c, dtype=np.float32))
    user_b = np.asarray(user_b, dtype=np.float32).reshape(-1)
    item_b = np.asarray(item_b, dtype=np.float32).reshape(-1)
    pop_item = np.asarray(pop_item, dtype=np.float32).reshape(-1)

    batch = user.shape[0]

    # ---- route to cores by user range; sort by (ubkt, ibkt) segment --------
    core_of = (user // UPC).astype(np.int32)
    local_u = (user - core_of.astype(np.int64) * UPC).astype(np.int32)
    seg_of = ((local_u >> 15) << 2 | (item >> 15)).astype(np.int32)  # 0..15

    # per (core, seg) element lists
    order = np.lexsort((seg_of, core_of))  # grouped by core, then seg
    core_sorted = core_of[order]
    seg_sorted = seg_of[order]

    counts = np.zeros((N_CORES, 16), dtype=np.int64)
    np.add.at(counts, (core_of, seg_of), 1)

    # padded segment sizes shared by all cores (multiples of 128)
    seg_pad = ((counts.max(axis=0) + 127) // 128 * 128).astype(np.int64)

    # chunk plan (identical across cores)
    chunks_meta = []  # (seg, n, t, off_within_seg)
    seg_off = np.zeros(17, dtype=np.int64)
    for sgi in range(16):
        seg_off[sgi + 1] = seg_off[sgi] + seg_pad[sgi]
        done = 0
        while done < seg_pad[sgi]:
            n = int(min(CHUNK, seg_pad[sgi] - done))
            chunks_meta.append((sgi, n, n // P, done))
            done += n
    n_pad = int(seg_off[16])
    c_all = 2 * (n_pad // 16)
    c_out = n_pad // P

    # device chunk list with buffer offsets
    chunks = []
    ucol = 0
    vcol = n_pad // 16
    ocol = 0
    for (sgi, n, t, _off) in chunks_meta:
        ub = (sgi >> 2) * BKT
        vb = (sgi & 3) * BKT
        chunks.append((n, t, ub, vb, ucol, vcol, ocol))
        ucol += n // 16
        vcol += n // 16
        ocol += t

    # ---- augmented 512B-row tables (pure concatenation) --------------------
    v_aug = np.zeros((NUM_ITEMS, ROWW), dtype=np.float32)
    v_aug[:, :EMBED] = item_e_c
    v_aug[:, EMBED] = pop_item
    v_aug[:, EMBED + 1] = item_b

    # ---- per-core index streams -------------------------------------------
    in_maps = []
    recon = []  # per core: list of (lo, hi) element ranges in `order`
    core_starts = np.searchsorted(core_sorted, np.arange(N_CORES + 1))
    for c in range(N_CORES):
        u_aug = np.zeros((UPC, ROWW), dtype=np.float32)
        u_aug[:, :EMBED] = user_e[c * UPC : (c + 1) * UPC]
        u_aug[:, EMBED] = user_b[c * UPC : (c + 1) * UPC]

        clo, chi = core_starts[c], core_starts[c + 1]
        segs = seg_sorted[clo:chi]
        seg_bounds = np.searchsorted(segs, np.arange(17))

        u_rel = np.zeros(n_pad, dtype=np.int16)
        v_rel = np.zeros(n_pad, dtype=np.int16)
        for sgi in range(16):
            lo, hi = seg_bounds[sgi], seg_bounds[sgi + 1]
            els = order[clo + lo : clo + hi]
            base = seg_off[sgi]
            cnt = hi - lo
            u_rel[base : base + cnt] = (local_u[els] - (sgi >> 2) * BKT).astype(
                np.int16
            )
            v_rel[base : base + cnt] = (item[els] - (sgi & 3) * BKT).astype(np.int16)
            # pad slots keep 0 (valid row at the bucket base)
        idx_arr = np.empty((P, c_all), dtype=np.int16)
        idx_arr[:, : n_pad // 16] = _wrap_idx(u_rel)
        idx_arr[:, n_pad // 16 :] = _wrap_idx(v_rel)

        in_maps.append({"u_tab": u_aug, "v_tab": v_aug, "idxs": idx_arr})
        recon.append((clo, chi, seg_bounds))

    nc = _build_program(chunks, c_all, c_out)

    if os.environ.get("KERNEL_SIM", "0") == "1":
        res = _run_sim(nc, in_maps)
    else:
        from concourse.bass_utils import run_bass_kernel_spmd

        trace = os.environ.get("KERNEL_TRACE", "0") == "1"
        res = run_bass_kernel_spmd(
            nc,
            in_maps,
            core_ids=list(range(N_CORES)),
            trace=trace,
        )
    LAST_EXEC_NS = res.exec_time_ns
    LAST_RESULTS = res

    # ---- reconstruct -------------------------------------------------------
    out_full = np.empty(batch, dtype=np.float32)
    for c in range(N_CORES):
        clo, chi, seg_bounds = recon[c]
        arr = res.results[c]["outd"]  # [P, c_out]
        flat = arr.reshape(P, c_out).ravel(order="F")  # element i at i (p fastest)
        for sgi in range(16):
            lo, hi = seg_bounds[sgi], seg_bounds[sgi + 1]
            els = order[clo + lo : clo + hi]
            base = int(seg_off[sgi])
            out_full[els] = flat[base : base + (hi - lo)]
    return out_full

